# revision 1
# baseline (speedup 1.0000x reference)
"""Bass/Trainium2 kernel for a 2-layer GCN with knowledge-enhanced output
(nn_KeGNN): y = log_softmax(relu(GCN2(relu(GCN1(x))) + P*K*U)).

Distribution strategy (8 NeuronCores, SPMD one NEFF):
  * Nodes are partitioned into 8 contiguous shards (12500 each); core c owns
    the edges whose *destination* is in shard c and produces the output rows
    of its shard.
  * GCN normalization is folded node-wise: with dinv = 1/sqrt(deg),
    table = dinv * (H @ W) gives messages, and the aggregated sum is scaled
    by dinv[dst].  The per-edge segment-sum becomes:
       agg[dst-tile] += S.T @ G        (TensorE matmul, PSUM accumulate)
    where G = dma_gather(table, src-index) and S is a 0/1 selection matrix
    built on VectorE with one is_equal against a static iota row.
  * Layer-1 table (dinv * (x @ W1), all 100k nodes) is computed redundantly
    on every core from a transposed copy of x -- cheaper than collectives.
  * Layer-2 table (dinv * (h1 @ W2), padded 40->64) is computed per-shard and
    AllGathered between the layers.
  * Source indices are int16 (hardware gather limit 32767) so the gather is
    split into 4 source blocks of 25000 nodes; per (dst-tile, block) segments
    are padded to multiples of 128 tokens, identically across cores so one
    program serves all 8 cores (per-core behavior differs only through the
    per-core index/dstloc input arrays).
"""

import numpy as np


# ----------------------------------------------------------------- config --
class CFG:
    N = 100000      # nodes
    F = 128         # input feature dim
    H = 64          # hidden dim
    O = 40          # output dim
    E = 1600000     # edges (without self loops)
    C = 8           # cores
    NBLK = 4        # src blocks (int16 gather index limit)
    CH_KT = 8       # K-tiles (of 128 tokens) per dma_gather call
                    # (SWDGE ucode ring: one call must be <= 1024 descriptors)
    SLAB = 2048     # nodes per xT slab load in table1 build
    DMA_SCRATCH = 16384   # per-partition SWDGE desc-ring carveout bytes
    STG = 14        # dst-tiles per staged DRAM write in postproc

    def __init__(self, **kw):
        for k, v in kw.items():
            setattr(self, k, v)
        assert self.N % self.C == 0
        self.SHARD = self.N // self.C
        self.NT = -(-self.SHARD // 128)          # dst tiles per core
        self.LASTV = self.SHARD - (self.NT - 1) * 128  # valid rows in last tile
        assert self.N % self.NBLK == 0
        self.BLK = self.N // self.NBLK
        assert self.BLK <= 32767
        self.NBT = -(-self.BLK // 128)           # node tiles per block
        self.HP = 64                             # padded layer-2 table width
        assert self.O <= self.HP


def _cdiv(a, b):
    return -(-a // b)


# ----------------------------------------------------- host preprocessing --
class Layout:
    """Cross-core-common token layout.

    Tokens are grouped by (src-block b, dst-supertile T, dst-tile t); each
    (b, t) group gets the cross-core max token count (ctok), supertile
    streams are padded to multiples of 128 so K-tiles never span supertiles.
    dstloc values are relative to the supertile base (< GT*128).
    """

    GT = 16  # dst tiles per supertile

    def __init__(self, cfg: CFG, ctok):
        self.ctok = ctok  # [NBLK, NT] common per-(b,t) token counts
        NT, NBLK = cfg.NT, cfg.NBLK
        self.NSUP = _cdiv(NT, self.GT)
        self.off = np.zeros((NBLK, NT), dtype=np.int64)  # global token offset
        self.nk_sup = np.zeros((NBLK, self.NSUP), dtype=np.int64)
        self.blk_kt_base = [0] * (NBLK + 1)
        pos = 0
        for b in range(NBLK):
            for T in range(self.NSUP):
                t0, t1 = T * self.GT, min((T + 1) * self.GT, NT)
                sup_len = 0
                for t in range(t0, t1):
                    self.off[b, t] = pos + sup_len
                    sup_len += int(ctok[b, t])
                sup_pad = _cdiv(sup_len, 128) * 128
                self.nk_sup[b, T] = sup_pad // 128
                pos += sup_pad
            self.blk_kt_base[b + 1] = pos // 128
        self.nktot = pos // 128
        self.ntok = pos


def _preprocess(edge_index, cfg: CFG):
    """Partition/sort edges, compute degrees, build per-core gather indices.

    Returns (deg, layout, per_core)."""
    N, C, NBLK = cfg.N, cfg.C, cfg.NBLK
    NT, SHARD, BLK = cfg.NT, cfg.SHARD, cfg.BLK

    loops = np.arange(N, dtype=np.int64)
    src = np.concatenate([np.asarray(edge_index[0], dtype=np.int64), loops])
    dst = np.concatenate([np.asarray(edge_index[1], dtype=np.int64), loops])
    deg = np.bincount(dst, minlength=N).astype(np.float32)

    core = dst // SHARD
    tloc = (dst % SHARD) // 128
    blk = src // BLK
    key = (core * NBLK + blk) * NT + tloc
    order = np.argsort(key, kind="stable")
    s_src = src[order]
    s_dst = dst[order]

    ngroups = C * NBLK * NT
    cnt = np.bincount(key, minlength=ngroups).reshape(C, NBLK, NT)
    starts = np.zeros(ngroups + 1, dtype=np.int64)
    np.cumsum(cnt.reshape(-1), out=starts[1:])

    lay = Layout(cfg, cnt.max(axis=0))
    GT = lay.GT

    per_core = []
    for c in range(C):
        idx_stream = np.zeros(lay.ntok, dtype=np.int16)
        dloc_stream = np.full(lay.ntok, 9999.0, dtype=np.float32)
        for b in range(NBLK):
            for t in range(NT):
                g = (c * NBLK + b) * NT + t
                a, e = starts[g], starts[g + 1]
                n = e - a
                pos = lay.off[b, t]
                idx_stream[pos:pos + n] = (s_src[a:e] - b * BLK).astype(np.int16)
                dloc_stream[pos:pos + n] = (
                    s_dst[a:e] - (c * SHARD + (t // GT) * GT * 128)
                ).astype(np.float32)
        idx_rep = np.ascontiguousarray(
            np.tile(idx_stream.reshape(-1, 16).T, (8, 1))
        )  # [128, ntok//16]
        dloc_w = np.ascontiguousarray(
            dloc_stream.reshape(-1, 128).T
        ).astype(np.float16)  # [128, nktot]
        per_core.append({"idx": idx_rep, "dloc": dloc_w})

    return deg, lay, per_core


def _wrap_deg(deg, cfg: CFG):
    """degB [128, NBLK*NBT] (block-wrapped, pad 1.0) and per-core degS
    [128, NT] (shard-wrapped, pad 1.0)."""
    N, NBLK, BLK, NBT = cfg.N, cfg.NBLK, cfg.BLK, cfg.NBT
    C, SHARD, NT = cfg.C, cfg.SHARD, cfg.NT
    degB = np.ones((128, NBLK * NBT), dtype=np.float32)
    for b in range(NBLK):
        for j in range(NBT):
            base = b * BLK + j * 128
            m = min(128, (b + 1) * BLK - base, N - base)
            if m > 0:
                degB[:m, b * NBT + j] = deg[base:base + m]
    degS = np.ones((C, 128, NT), dtype=np.float32)
    for c in range(C):
        for t in range(NT):
            base = c * SHARD + t * 128
            m = min(128, (c + 1) * SHARD - base)
            degS[c, :m, t] = deg[base:base + m]
    return degB, degS


# ------------------------------------------------------------ bass program --
def _build(cfg: CFG, lay: Layout):
    import concourse.bacc as bacc
    import concourse.mybir as mybir
    from concourse import tile

    f32 = mybir.dt.float32
    f16 = mybir.dt.float16
    i16 = mybir.dt.int16
    i32 = mybir.dt.int32
    ALU = mybir.AluOpType
    ACTF = mybir.ActivationFunctionType

    N, F, H, O, C = cfg.N, cfg.F, cfg.H, cfg.O, cfg.C
    NBLK, BLK, NBT = cfg.NBLK, cfg.BLK, cfg.NBT
    NT, SHARD, LASTV, HP = cfg.NT, cfg.SHARD, cfg.LASTV, cfg.HP
    CH_KT, SLAB, STG = cfg.CH_KT, cfg.SLAB, cfg.STG

    nktot = lay.nktot
    ntok = lay.ntok
    blk_kt_base = lay.blk_kt_base
    GT = lay.GT

    nc = bacc.Bacc("TRN2", target_bir_lowering=False, debug=False,
                   num_devices=cfg.C,
                   dynamic_dma_scratch_size=cfg.DMA_SCRATCH,
                   num_swdge_queues=4)

    # ---- DRAM I/O
    xT_d = nc.dram_tensor("xT", [F, N], f32, kind="ExternalInput")
    degB_d = nc.dram_tensor("degB", [128, NBLK * NBT], f32, kind="ExternalInput")
    degS_d = nc.dram_tensor("degS", [128, NT], f32, kind="ExternalInput")
    idx_d = nc.dram_tensor("idx", [128, ntok // 16], i16, kind="ExternalInput")
    dloc_d = nc.dram_tensor("dloc", [128, nktot], f16, kind="ExternalInput")
    W1_d = nc.dram_tensor("W1", [F, H], f32, kind="ExternalInput")
    W2_d = nc.dram_tensor("W2", [H, O], f32, kind="ExternalInput")
    b1_d = nc.dram_tensor("b1", [1, H], f32, kind="ExternalInput")
    b2_d = nc.dram_tensor("b2", [1, O], f32, kind="ExternalInput")
    P_d = nc.dram_tensor("P", [1, O], f32, kind="ExternalInput")
    K_d = nc.dram_tensor("K", [1, O], f32, kind="ExternalInput")
    U_d = nc.dram_tensor("U", [1, O], f32, kind="ExternalInput")
    out_d = nc.dram_tensor("out", [SHARD, O], f32, kind="ExternalOutput")

    TW = 128  # f16 table row width (256B gather granule; cols >= H unused)
    tab1 = [
        nc.dram_tensor(f"tab1_{b}", [min(BLK, N - b * BLK), TW], f16)
        for b in range(NBLK)
    ]
    t2loc = nc.dram_tensor("t2loc", [SHARD, TW], f16)
    tab2 = nc.dram_tensor("tab2", [N, TW], f16, addr_space="Shared")

    with tile.TileContext(nc, num_cores=C) as tc:
        with (
            tc.tile_pool(name="const", bufs=1) as const,
            tc.tile_pool(name="xslab", bufs=2) as xpool,
            tc.tile_pool(name="t1st", bufs=2) as t1pool,
            tc.tile_pool(name="g", bufs=8) as gpool,
            tc.tile_pool(name="s", bufs=4) as spool,
            tc.tile_pool(name="work", bufs=2) as work,
            tc.tile_pool(name="post", bufs=2) as post,
            tc.tile_pool(name="ost", bufs=2) as opool,
            tc.tile_pool(name="ps_seg", bufs=3, space="PSUM") as ps_seg,
            tc.tile_pool(name="ps_bld", bufs=2, space="PSUM") as ps_bld,
            tc.tile_pool(name="ps_tr", bufs=1, space="PSUM") as ps_tr,
            tc.tile_pool(name="ps_t2", bufs=1, space="PSUM") as ps_t2,
        ):
            # ---------------- constants / small inputs
            iota_i = const.tile([128, GT * 128], i32)
            nc.gpsimd.iota(iota_i[:, :], pattern=[[128, GT], [1, 128]],
                           base=0, channel_multiplier=0)
            IOTA16 = const.tile([128, GT * 128], f16)
            nc.vector.tensor_copy(IOTA16[:, :], iota_i[:, :])
            IOTA = IOTA16  # first 128 columns are a plain 0..127 iota row
            IDiota = const.tile([128, 128], f32)
            pidx_i = const.tile([128, 1], i32)
            nc.gpsimd.iota(pidx_i[:, :], pattern=[[0, 1]], base=0,
                           channel_multiplier=1)
            PIDX = const.tile([128, 1], f32)
            nc.vector.tensor_copy(PIDX[:, :], pidx_i[:, :])
            ID = const.tile([128, 128], f32)
            nc.vector.tensor_copy(IDiota[:, :], iota_i[:, :128])
            nc.vector.tensor_scalar(out=ID[:, :], in0=IDiota[:, :],
                                    scalar1=PIDX[:, :], scalar2=None,
                                    op0=ALU.is_equal)

            W1s = const.tile([F, H], f32)
            nc.sync.dma_start(W1s[:, :], W1_d[:, :])
            W2s = const.tile([H, O], f32)
            nc.sync.dma_start(W2s[:, :], W2_d[:, :])

            b1row = const.tile([1, H], f32)
            nc.sync.dma_start(b1row[:, :], b1_d[:, :])
            BIAS1 = const.tile([128, H], f32)
            nc.gpsimd.partition_broadcast(BIAS1[:, :], b1row[:, :])

            b2row = const.tile([1, O], f32)
            nc.sync.dma_start(b2row[:, :], b2_d[:, :])
            prow = const.tile([1, O], f32)
            nc.sync.dma_start(prow[:, :], P_d[:, :])
            krow = const.tile([1, O], f32)
            nc.sync.dma_start(krow[:, :], K_d[:, :])
            urow = const.tile([1, O], f32)
            nc.sync.dma_start(urow[:, :], U_d[:, :])
            pku = const.tile([1, O], f32)
            nc.vector.tensor_mul(pku[:, :], prow[:, :], krow[:, :])
            nc.vector.tensor_mul(pku[:, :], pku[:, :], urow[:, :])
            nc.vector.tensor_add(pku[:, :], pku[:, :], b2row[:, :])
            BIAS2 = const.tile([128, O], f32)
            nc.gpsimd.partition_broadcast(BIAS2[:, :], pku[:, :])

            degB = const.tile([128, NBLK * NBT], f32)
            nc.sync.dma_start(degB[:, :], degB_d[:, :])
            dinvB = const.tile([128, NBLK * NBT], f32)
            nc.vector.reciprocal(dinvB[:, :], degB[:, :])
            nc.scalar.sqrt(dinvB[:, :], dinvB[:, :])

            degS = const.tile([128, NT], f32)
            nc.sync.dma_start(degS[:, :], degS_d[:, :])
            dinvS = const.tile([128, NT], f32)
            nc.vector.reciprocal(dinvS[:, :], degS[:, :])
            nc.scalar.sqrt(dinvS[:, :], dinvS[:, :])

            idxS = const.tile([128, ntok // 16], i16)
            nc.sync.dma_start(idxS[:, :], idx_d[:, :])
            dloc = const.tile([128, nktot], f16)
            nc.sync.dma_start(dloc[:, :], dloc_d[:, :])

            agg = const.tile([128, NT, H], f32)
            nc.vector.memset(agg[:, :, :], 0.0)

            # ---------------- layer-1 message table: tab1_b = dinv*(x@W1)
            def build_table1(b):
                nodes_b = min(BLK, N - b * BLK)
                for s0 in range(0, nodes_b, SLAB):
                    w = min(SLAB, nodes_b - s0)
                    xs = xpool.tile([F, SLAB], f32, tag="xs")
                    nc.sync.dma_start(xs[:, :w],
                                      xT_d[:, b * BLK + s0: b * BLK + s0 + w])
                    st = t1pool.tile([128, _cdiv(SLAB, 128), H], f16, tag="t1st")
                    nfull = 0
                    for j0 in range(0, w, 128):
                        m = min(128, w - j0)
                        jt = (s0 + j0) // 128  # node-tile idx within block
                        ps = ps_bld.tile([128, H], f32, tag="psb")
                        nc.tensor.matmul(ps[:m, :], lhsT=xs[:, j0:j0 + m],
                                         rhs=W1s[:, :], start=True, stop=True)
                        nc.scalar.activation(
                            st[:m, j0 // 128, :], ps[:m, :], ACTF.Copy,
                            scale=dinvB[:m, b * NBT + jt: b * NBT + jt + 1])
                        if m == 128:
                            nfull += 1
                    # store staged tiles to DRAM
                    if nfull:
                        dst_ap = tab1[b][s0:s0 + nfull * 128, :H].rearrange(
                            "(j p) f -> p j f", p=128)
                        nc.sync.dma_start(dst_ap, st[:, :nfull, :])
                    if nfull * 128 < w:  # ragged tail tile of the block
                        m = w - nfull * 128
                        nc.sync.dma_start(
                            tab1[b][s0 + nfull * 128: s0 + w, :H],
                            st[:m, nfull, :])

            for b in range(NBLK):
                build_table1(b)

            # ---------------- gather + segment-sum matmul for one layer
            MAXKB = 8  # S-matrices built per DVE instruction
            qrot = [0]  # SWDGE queue rotation across gather calls

            def seg_layer(table_aps, uw):
                """table_aps[b]: block b's [rows, TW] f16 message rows; only
                the first uw columns are meaningful."""
                for b in range(NBLK):
                    kt_in_blk = blk_kt_base[b + 1] - blk_kt_base[b]
                    if kt_in_blk == 0:
                        continue
                    # gather chunks
                    gtiles = []
                    for ci in range(_cdiv(kt_in_blk, CH_KT)):
                        kts = min(CH_KT, kt_in_blk - ci * CH_KT)
                        g = gpool.tile([128, CH_KT, TW], f16, tag="g")
                        tok0 = (blk_kt_base[b] + ci * CH_KT) * 128
                        nc.gpsimd.dma_gather(
                            g[:, :kts, :], table_aps[b],
                            idxS[:, tok0 // 16: (tok0 + kts * 128) // 16],
                            num_idxs=kts * 128, num_idxs_reg=kts * 128,
                            elem_size=TW, single_packet=False,
                            queue_num=qrot[0] % 4)
                        qrot[0] += 1
                        gtiles.append(g)

                    def gslice(kglob):
                        ci, sl = divmod(kglob - blk_kt_base[b], CH_KT)
                        return gtiles[ci][:, sl, :uw]

                    # consume: per dst-tile, its token range [o0, o1) in the
                    # common layout; K-tiles at supertile boundaries are
                    # shared between adjacent dst-tiles (S masks the others).
                    for t in range(NT):
                        ct = int(lay.ctok[b, t])
                        if ct == 0:
                            continue
                        o0 = int(lay.off[b, t])
                        o1 = o0 + ct
                        k0, k1 = o0 // 128, (o1 - 1) // 128
                        it = t % GT  # iota variant within supertile
                        ps = ps_seg.tile([128, uw], f32, tag="pss")
                        k = k0
                        while k <= k1:
                            kb = min(MAXKB, k1 + 1 - k)
                            Sb = spool.tile([128, MAXKB, 128], f16, tag="s")
                            nc.vector.tensor_tensor(
                                out=Sb[:, :kb, :],
                                in0=IOTA16[:, it * 128:(it + 1) * 128]
                                    .unsqueeze(1)
                                    .broadcast_to([128, kb, 128]),
                                in1=dloc[:, k:k + kb].unsqueeze(2)
                                    .broadcast_to([128, kb, 128]),
                                op=ALU.is_equal)
                            for j in range(kb):
                                nc.tensor.matmul(
                                    ps[:, :], lhsT=Sb[:, j, :],
                                    rhs=gslice(k + j),
                                    start=(k + j == k0),
                                    stop=(k + j == k1))
                            k += kb
                        nc.vector.tensor_add(agg[:, t, :uw],
                                             agg[:, t, :uw], ps[:, :])

            # ---------------- layer 1
            tab1_aps = [tab1[b][:, :] for b in range(NBLK)]
            seg_layer(tab1_aps, H)

            # post: h1 = relu(dinv*agg + b1); t2 = dinv*(h1@W2) padded
            def staged_store(dram, stile, grp, nt_in_grp, width):
                """store staging tile rows [grp*STG .. ) handling ragged tail"""
                t0 = grp * STG
                nfull = 0
                for tt in range(nt_in_grp):
                    if (t0 + tt) * 128 + 128 <= SHARD:
                        nfull += 1
                if nfull:
                    dst = dram[t0 * 128: t0 * 128 + nfull * 128,
                               :width].rearrange("(j p) f -> p j f", p=128)
                    nc.sync.dma_start(dst, stile[:, :nfull, :width])
                if nfull < nt_in_grp:
                    nc.sync.dma_start(
                        dram[(t0 + nfull) * 128: SHARD, :width],
                        stile[:LASTV, nfull, :width])

            for grp in range(_cdiv(NT, STG)):
                nt_in_grp = min(STG, NT - grp * STG)
                st = post.tile([128, STG, H], f16, tag="t2st")
                if H > O:
                    nc.vector.memset(st[:, :, O:], 0.0)
                for tt in range(nt_in_grp):
                    t = grp * STG + tt
                    h1 = work.tile([128, H], f32, tag="h1")
                    nc.vector.scalar_tensor_tensor(
                        out=h1[:, :], in0=agg[:, t, :],
                        scalar=dinvS[:, t:t + 1], in1=BIAS1[:, :],
                        op0=ALU.mult, op1=ALU.add)
                    nc.scalar.activation(h1[:, :], h1[:, :], ACTF.Relu)
                    pst = ps_tr.tile([H, 128], f32, tag="pstr")
                    nc.tensor.transpose(pst[:, :], h1[:, :], ID[:, :])
                    h1t = work.tile([H, 128], f32, tag="h1t")
                    nc.scalar.copy(h1t[:, :], pst[:, :])
                    ps2 = ps_t2.tile([128, O], f32, tag="pst2")
                    nc.tensor.matmul(ps2[:, :], lhsT=h1t[:, :], rhs=W2s[:, :],
                                     start=True, stop=True)
                    nc.scalar.activation(st[:, tt, :O], ps2[:, :], ACTF.Copy,
                                         scale=dinvS[:, t:t + 1])
                staged_store(t2loc, st, grp, nt_in_grp, H)

            # ---------------- exchange layer-2 table
            nc.gpsimd.collective_compute(
                "AllGather", mybir.AluOpType.bypass,
                replica_groups=[list(range(C))],
                ins=[t2loc[:, :].opt()],
                outs=[tab2[:, :].opt()])

            # ---------------- layer 2
            nc.vector.memset(agg[:, :, :], 0.0)
            tab2_aps = [tab2[b * BLK: b * BLK + min(BLK, N - b * BLK), :]
                        for b in range(NBLK)]
            seg_layer(tab2_aps, O)

            # post: y = relu(dinv*agg + b2 + pku); out = log_softmax(y)
            for grp in range(_cdiv(NT, STG)):
                nt_in_grp = min(STG, NT - grp * STG)
                st = opool.tile([128, STG, O], f32, tag="ost")
                for tt in range(nt_in_grp):
                    t = grp * STG + tt
                    y = work.tile([128, O], f32, tag="y")
                    nc.vector.scalar_tensor_tensor(
                        out=y[:, :], in0=agg[:, t, :O],
                        scalar=dinvS[:, t:t + 1], in1=BIAS2[:, :],
                        op0=ALU.mult, op1=ALU.add)
                    nc.scalar.activation(y[:, :], y[:, :], ACTF.Relu)
                    nmax = work.tile([128, 1], f32, tag="nmax")
                    nc.vector.tensor_reduce(nmax[:, :], y[:, :],
                                            axis=mybir.AxisListType.X,
                                            op=ALU.max, negate=True)
                    ex = work.tile([128, O], f32, tag="ex")
                    esum = work.tile([128, 1], f32, tag="esum")
                    nc.scalar.activation(ex[:, :], y[:, :], ACTF.Exp,
                                         bias=nmax[:, :], scale=1.0,
                                         accum_out=esum[:, :])
                    lsum = work.tile([128, 1], f32, tag="lsum")
                    nc.scalar.activation(lsum[:, :], esum[:, :], ACTF.Ln)
                    nc.vector.tensor_scalar(
                        out=st[:, tt, :], in0=y[:, :], scalar1=nmax[:, :],
                        scalar2=lsum[:, :], op0=ALU.add, op1=ALU.subtract)
                staged_store(out_d, st, grp, nt_in_grp, O)

    nc.compile()
    return nc


# ------------------------------------------------------------------ entry --
def prepare_and_run(inputs, cfg=None, trace=False, **run_kwargs):
    """Preprocess, build, run on 8 cores.  Returns (out, BassKernelResults)."""
    from concourse.bass_utils import run_bass_kernel_spmd

    cfg = cfg or CFG()
    x = np.asarray(inputs["x"], dtype=np.float32)
    edge_index = np.asarray(inputs["edge_index"])
    W1 = np.asarray(inputs["W1"], dtype=np.float32)
    b1 = np.asarray(inputs["b1"], dtype=np.float32)
    W2 = np.asarray(inputs["W2"], dtype=np.float32)
    b2 = np.asarray(inputs["b2"], dtype=np.float32)
    P = np.asarray(inputs["P"], dtype=np.float32)
    K = np.asarray(inputs["K"], dtype=np.float32)
    U = np.asarray(inputs["U"], dtype=np.float32)

    deg, lay, per_core = _preprocess(edge_index, cfg)
    degB, degS = _wrap_deg(deg, cfg)
    xT = np.ascontiguousarray(x.T)

    nc = _build(cfg, lay)

    in_maps = []
    for c in range(cfg.C):
        in_maps.append({
            "xT": xT,
            "degB": degB,
            "degS": np.ascontiguousarray(degS[c]),
            "idx": per_core[c]["idx"],
            "dloc": per_core[c]["dloc"],
            "W1": W1, "W2": W2,
            "b1": b1.reshape(1, -1), "b2": b2.reshape(1, -1),
            "P": P.reshape(1, -1), "K": K.reshape(1, -1),
            "U": U.reshape(1, -1),
        })

    res = run_bass_kernel_spmd(nc, in_maps, core_ids=list(range(cfg.C)),
                               trace=trace, **run_kwargs)
    out = np.concatenate([res.results[c]["out"] for c in range(cfg.C)], axis=0)
    return out.astype(np.float32), res


def kernel(**inputs):
    out, _ = prepare_and_run(inputs)
    return out


if __name__ == "__main__":
    import reference

    inputs = {k: np.asarray(v) for k, v in reference.setup_inputs().items()}
    got = kernel(**inputs)
    want = np.asarray(reference.reference(**inputs))
    err = np.abs(got - want).max() / max(np.abs(want).max(), 1e-9)
    print("rel err:", err)



# revision 13
# speedup vs baseline: 1.3804x; 1.3804x over previous
"""Bass/Trainium2 kernel for a 2-layer GCN with knowledge-enhanced output
(nn_KeGNN): y = log_softmax(relu(GCN2(relu(GCN1(x))) + P*K*U)).

Distribution strategy (8 NeuronCores, SPMD one NEFF):
  * Nodes are partitioned into 8 contiguous shards (12500 each); core c owns
    the edges whose *destination* is in shard c and produces the output rows
    of its shard.
  * GCN normalization is folded node-wise: with dinv = 1/sqrt(deg),
    table = dinv * (H @ W) gives messages, and the aggregated sum is scaled
    by dinv[dst].  The per-edge segment-sum becomes:
       agg[dst-tile] += S.T @ G        (TensorE matmul, PSUM accumulate)
    where G = dma_gather(table, src-index) and S is a 0/1 selection matrix
    built on VectorE with one is_equal against a static iota row.
  * Self-loops are NOT in the token stream: their contribution
    dinv[v]^2*(h@W)[v] is folded into a per-tile fused bias
    (FUSED = b + dinv^2 * (h_own @ W)), so the post-aggregation epilogue
    stays one scalar_tensor_tensor per tile.  This also removes the +128
    own-shard asymmetry that forced ~20% cross-core padding of the common
    token layout.
  * Layer-1 table (dinv * (x @ W1), all 100k nodes, f16 compute) is computed
    redundantly on every core from a transposed f16 copy of x -- cheaper
    than collectives.
  * Layer-2 table (dinv * (h1 @ W2), padded 40->64) is computed per-shard
    and AllGathered between the layers in 2 chunks (first chunk overlaps
    the rest of the layer-1 epilogue).
  * Source indices are int16 (hardware gather limit 32767) so the gather is
    split into 4 source blocks of 25000 nodes; per (dst-tile, block) segments
    are padded to multiples of 128 tokens, identically across cores so one
    program serves all 8 cores (per-core behavior differs only through the
    per-core index/dstloc/x_own input arrays).
"""

import numpy as np


# ----------------------------------------------------------------- config --
class CFG:
    N = 100000      # nodes
    F = 128         # input feature dim
    H = 64          # hidden dim
    O = 40          # output dim
    E = 1600000     # edges (without self loops)
    C = 8           # cores
    NBLK = 4        # src blocks (int16 gather index limit)
    CH_KT = 8       # K-tiles (of 128 tokens) per dma_gather call
                    # (SWDGE ucode ring: one call must be <= 1024 descriptors)
    SLAB = 2048     # nodes per xT slab load in table1 build
    DMA_SCRATCH = 16384   # per-partition SWDGE desc-ring carveout bytes
    STG = 14        # dst-tiles per staged DRAM write in postproc
    GBUF = 16       # gather tile-pool depth (in-flight gather chunks)
    AGCH = 1        # AllGather chunks (BIR: collective outs must be
                    # contiguous, so only 1 is currently valid)
    ABL = frozenset()  # ablation flags (experiments only; default none)

    def __init__(self, **kw):
        for k, v in kw.items():
            setattr(self, k, v)
        assert self.N % self.C == 0
        self.SHARD = self.N // self.C
        self.NT = -(-self.SHARD // 128)          # dst tiles per core
        self.LASTV = self.SHARD - (self.NT - 1) * 128  # valid rows last tile
        assert self.N % self.NBLK == 0
        self.BLK = self.N // self.NBLK
        assert self.BLK <= 32767
        self.NBT = -(-self.BLK // 128)           # node tiles per block
        self.HP = 64                             # padded layer-2 table width
        assert self.O <= self.HP


def _cdiv(a, b):
    return -(-a // b)


# ----------------------------------------------------- host preprocessing --
class Layout:
    """Cross-core-common token layout.

    Tokens are grouped by (src-block b, dst-supertile T, dst-tile t); each
    (b, t) group gets the cross-core max token count (ctok), supertile
    streams are padded to multiples of 128 so K-tiles never span supertiles.
    dstloc values are relative to the supertile base (< GT*128).
    """

    GT = 16  # dst tiles per supertile

    def __init__(self, cfg: CFG, ctok):
        self.ctok = ctok  # [NBLK, NT] common per-(b,t) token counts
        NT, NBLK = cfg.NT, cfg.NBLK
        self.NSUP = _cdiv(NT, self.GT)
        self.off = np.zeros((NBLK, NT), dtype=np.int64)  # global token offset
        self.nk_sup = np.zeros((NBLK, self.NSUP), dtype=np.int64)
        self.blk_kt_base = [0] * (NBLK + 1)
        pos = 0
        for b in range(NBLK):
            for T in range(self.NSUP):
                t0, t1 = T * self.GT, min((T + 1) * self.GT, NT)
                sup_len = 0
                for t in range(t0, t1):
                    self.off[b, t] = pos + sup_len
                    sup_len += int(ctok[b, t])
                sup_pad = _cdiv(sup_len, 128) * 128
                self.nk_sup[b, T] = sup_pad // 128
                pos += sup_pad
            self.blk_kt_base[b + 1] = pos // 128
        self.nktot = pos // 128
        self.ntok = pos


def _preprocess(edge_index, cfg: CFG):
    """Partition/sort edges, compute degrees, build per-core gather indices.

    Self-loops contribute to deg but are NOT in the token stream (handled
    analytically in the epilogue).  Returns (deg, layout, per_core)."""
    N, C, NBLK = cfg.N, cfg.C, cfg.NBLK
    NT, SHARD, BLK = cfg.NT, cfg.SHARD, cfg.BLK

    src = np.asarray(edge_index[0], dtype=np.int64)
    dst = np.asarray(edge_index[1], dtype=np.int64)
    deg = (np.bincount(dst, minlength=N) + 1).astype(np.float32)

    core = dst // SHARD
    tloc = (dst % SHARD) // 128
    blk = src // BLK
    key = (core * NBLK + blk) * NT + tloc
    order = np.argsort(key, kind="stable")
    s_src = src[order]
    s_dst = dst[order]

    ngroups = C * NBLK * NT
    cnt = np.bincount(key, minlength=ngroups).reshape(C, NBLK, NT)
    starts = np.zeros(ngroups + 1, dtype=np.int64)
    np.cumsum(cnt.reshape(-1), out=starts[1:])

    lay = Layout(cfg, cnt.max(axis=0))
    GT = lay.GT

    per_core = []
    for c in range(C):
        idx_stream = np.zeros(lay.ntok, dtype=np.int16)
        dloc_stream = np.full(lay.ntok, 9999.0, dtype=np.float32)
        for b in range(NBLK):
            for t in range(NT):
                g = (c * NBLK + b) * NT + t
                a, e = starts[g], starts[g + 1]
                n = e - a
                pos = lay.off[b, t]
                idx_stream[pos:pos + n] = (s_src[a:e] - b * BLK).astype(np.int16)
                dloc_stream[pos:pos + n] = (
                    s_dst[a:e] - (c * SHARD + (t // GT) * GT * 128)
                ).astype(np.float32)
        idx_rep = np.ascontiguousarray(
            np.tile(idx_stream.reshape(-1, 16).T, (8, 1))
        )  # [128, ntok//16]
        dloc_w = np.ascontiguousarray(
            dloc_stream.reshape(-1, 128).T
        ).astype(np.float16)  # [128, nktot]
        per_core.append({"idx": idx_rep, "dloc": dloc_w})

    return deg, lay, per_core


def _wrap_deg(deg, cfg: CFG):
    """degB [128, NBLK*NBT] (block-wrapped, pad 1.0) and per-core degS
    [128, NT] (shard-wrapped, pad 1.0)."""
    N, NBLK, BLK, NBT = cfg.N, cfg.NBLK, cfg.BLK, cfg.NBT
    C, SHARD, NT = cfg.C, cfg.SHARD, cfg.NT
    degB = np.ones((128, NBLK * NBT), dtype=np.float32)
    for b in range(NBLK):
        for j in range(NBT):
            base = b * BLK + j * 128
            m = min(128, (b + 1) * BLK - base, N - base)
            if m > 0:
                degB[:m, b * NBT + j] = deg[base:base + m]
    degS = np.ones((C, 128, NT), dtype=np.float32)
    for c in range(C):
        for t in range(NT):
            base = c * SHARD + t * 128
            m = min(128, (c + 1) * SHARD - base)
            degS[c, :m, t] = deg[base:base + m]
    return degB, degS


def host_prepare(inputs, cfg: CFG):
    """All host-side preprocessing.  Returns (lay, in_maps)."""
    x = np.asarray(inputs["x"], dtype=np.float32)
    edge_index = np.asarray(inputs["edge_index"])
    deg, lay, per_core = _preprocess(edge_index, cfg)
    degB, degS = _wrap_deg(deg, cfg)
    xT16 = np.ascontiguousarray(x.T.astype(np.float16))
    NTP = cfg.NT * 128
    in_maps = []
    for c in range(cfg.C):
        xo = np.zeros((cfg.F, NTP), dtype=np.float16)
        xo[:, :cfg.SHARD] = xT16[:, c * cfg.SHARD:(c + 1) * cfg.SHARD]
        in_maps.append({
            "xT": xT16,
            "xoT": xo,
            "degB": degB,
            "degS": np.ascontiguousarray(degS[c]),
            "idx": per_core[c]["idx"],
            "dloc": per_core[c]["dloc"],
            "W1": np.asarray(inputs["W1"], np.float16),
            "W2": np.asarray(inputs["W2"], np.float32),
            "b1": np.asarray(inputs["b1"], np.float32).reshape(1, -1),
            "b2": np.asarray(inputs["b2"], np.float32).reshape(1, -1),
            "P": np.asarray(inputs["P"], np.float32).reshape(1, -1),
            "K": np.asarray(inputs["K"], np.float32).reshape(1, -1),
            "U": np.asarray(inputs["U"], np.float32).reshape(1, -1),
        })
    return lay, in_maps


# ------------------------------------------------------------ bass program --
def _build(cfg: CFG, lay: Layout):
    import concourse.bacc as bacc
    import concourse.mybir as mybir
    from concourse import tile

    f32 = mybir.dt.float32
    f16 = mybir.dt.float16
    i16 = mybir.dt.int16
    i32 = mybir.dt.int32
    ALU = mybir.AluOpType
    ACTF = mybir.ActivationFunctionType

    N, F, H, O, C = cfg.N, cfg.F, cfg.H, cfg.O, cfg.C
    NBLK, BLK, NBT = cfg.NBLK, cfg.BLK, cfg.NBT
    NT, SHARD, LASTV, HP = cfg.NT, cfg.SHARD, cfg.LASTV, cfg.HP
    CH_KT, SLAB, STG = cfg.CH_KT, cfg.SLAB, cfg.STG

    nktot = lay.nktot
    ntok = lay.ntok
    blk_kt_base = lay.blk_kt_base
    GT = lay.GT

    nc = bacc.Bacc("TRN2", target_bir_lowering=False, debug=False,
                   num_devices=cfg.C,
                   dynamic_dma_scratch_size=cfg.DMA_SCRATCH,
                   num_swdge_queues=4)

    # ---- DRAM I/O
    xT_d = nc.dram_tensor("xT", [F, N], f16, kind="ExternalInput")
    xoT_d = nc.dram_tensor("xoT", [F, NT * 128], f16, kind="ExternalInput")
    degB_d = nc.dram_tensor("degB", [128, NBLK * NBT], f32, kind="ExternalInput")
    degS_d = nc.dram_tensor("degS", [128, NT], f32, kind="ExternalInput")
    idx_d = nc.dram_tensor("idx", [128, ntok // 16], i16, kind="ExternalInput")
    dloc_d = nc.dram_tensor("dloc", [128, nktot], f16, kind="ExternalInput")
    W1_d = nc.dram_tensor("W1", [F, H], f16, kind="ExternalInput")
    W2_d = nc.dram_tensor("W2", [H, O], f32, kind="ExternalInput")
    b1_d = nc.dram_tensor("b1", [1, H], f32, kind="ExternalInput")
    b2_d = nc.dram_tensor("b2", [1, O], f32, kind="ExternalInput")
    P_d = nc.dram_tensor("P", [1, O], f32, kind="ExternalInput")
    K_d = nc.dram_tensor("K", [1, O], f32, kind="ExternalInput")
    U_d = nc.dram_tensor("U", [1, O], f32, kind="ExternalInput")
    out_d = nc.dram_tensor("out", [SHARD, O], f32, kind="ExternalOutput")

    TW = 128  # f16 table row width (256B gather granule; cols >= H unused)
    tab1 = [
        nc.dram_tensor(f"tab1_{b}", [min(BLK, N - b * BLK), TW], f16)
        for b in range(NBLK)
    ]
    t2loc = nc.dram_tensor("t2loc", [SHARD, TW], f16)
    tab2 = nc.dram_tensor("tab2", [N, TW], f16, addr_space="Shared")

    with tile.TileContext(nc, num_cores=C) as tc:
        with (
            tc.tile_pool(name="const", bufs=1) as const,
            tc.tile_pool(name="xslab", bufs=2) as xpool,
            tc.tile_pool(name="t1st", bufs=2) as t1pool,
            tc.tile_pool(name="g", bufs=cfg.GBUF) as gpool,
            tc.tile_pool(name="s", bufs=4) as spool,
            tc.tile_pool(name="work", bufs=2) as work,
            tc.tile_pool(name="post", bufs=2) as post,
            tc.tile_pool(name="ost", bufs=2) as opool,
            tc.tile_pool(name="ps_seg", bufs=3, space="PSUM") as ps_seg,
            tc.tile_pool(name="ps_bld", bufs=2, space="PSUM") as ps_bld,
            tc.tile_pool(name="ps_tr", bufs=1, space="PSUM") as ps_tr,
            tc.tile_pool(name="ps_t2", bufs=1, space="PSUM") as ps_t2,
        ):
            # ---------------- constants / small inputs
            iota_i = const.tile([128, GT * 128], i32)
            nc.gpsimd.iota(iota_i[:, :], pattern=[[128, GT], [1, 128]],
                           base=0, channel_multiplier=0)
            IOTA16 = const.tile([128, GT * 128], f16)
            nc.vector.tensor_copy(IOTA16[:, :], iota_i[:, :])
            IDiota = const.tile([128, 128], f32)
            pidx_i = const.tile([128, 1], i32)
            nc.gpsimd.iota(pidx_i[:, :], pattern=[[0, 1]], base=0,
                           channel_multiplier=1)
            PIDX = const.tile([128, 1], f32)
            nc.vector.tensor_copy(PIDX[:, :], pidx_i[:, :])
            ID = const.tile([128, 128], f32)
            nc.vector.tensor_copy(IDiota[:, :], iota_i[:, :128])
            nc.vector.tensor_scalar(out=ID[:, :], in0=IDiota[:, :],
                                    scalar1=PIDX[:, :], scalar2=None,
                                    op0=ALU.is_equal)

            W1s = const.tile([F, H], f16)
            nc.sync.dma_start(W1s[:, :], W1_d[:, :])
            W2s = const.tile([H, O], f32)
            nc.sync.dma_start(W2s[:, :], W2_d[:, :])

            b1row = const.tile([1, H], f32)
            nc.sync.dma_start(b1row[:, :], b1_d[:, :])
            BIAS1 = const.tile([128, H], f32)
            nc.gpsimd.partition_broadcast(BIAS1[:, :], b1row[:, :])

            b2row = const.tile([1, O], f32)
            nc.sync.dma_start(b2row[:, :], b2_d[:, :])
            prow = const.tile([1, O], f32)
            nc.sync.dma_start(prow[:, :], P_d[:, :])
            krow = const.tile([1, O], f32)
            nc.sync.dma_start(krow[:, :], K_d[:, :])
            urow = const.tile([1, O], f32)
            nc.sync.dma_start(urow[:, :], U_d[:, :])
            pku = const.tile([1, O], f32)
            nc.vector.tensor_mul(pku[:, :], prow[:, :], krow[:, :])
            nc.vector.tensor_mul(pku[:, :], pku[:, :], urow[:, :])
            nc.vector.tensor_add(pku[:, :], pku[:, :], b2row[:, :])
            BIAS2 = const.tile([128, O], f32)
            nc.gpsimd.partition_broadcast(BIAS2[:, :], pku[:, :])

            degB = const.tile([128, NBLK * NBT], f32)
            nc.sync.dma_start(degB[:, :], degB_d[:, :])
            dinvB = const.tile([128, NBLK * NBT], f32)
            nc.vector.reciprocal(dinvB[:, :], degB[:, :])
            nc.scalar.sqrt(dinvB[:, :], dinvB[:, :])

            degS = const.tile([128, NT], f32)
            nc.sync.dma_start(degS[:, :], degS_d[:, :])
            dinvS = const.tile([128, NT], f32)
            nc.vector.reciprocal(dinvS[:, :], degS[:, :])
            nc.scalar.sqrt(dinvS[:, :], dinvS[:, :])
            dinvS2 = const.tile([128, NT], f32)  # dinv^2 = 1/deg
            nc.vector.reciprocal(dinvS2[:, :], degS[:, :])

            idxS = const.tile([128, ntok // 16], i16)
            nc.sync.dma_start(idxS[:, :], idx_d[:, :])
            dloc = const.tile([128, nktot], f16)
            nc.sync.dma_start(dloc[:, :], dloc_d[:, :])

            agg = const.tile([128, NT, H], f32)
            nc.vector.memset(agg[:, :, :], 0.0)

            # ---------------- fused layer-1 bias:
            #   FUSED1[:, t, :] = b1 + dinv^2 * (x_own @ W1)   (self-loop term)
            xoS = const.tile([F, NT * 128], f16)
            nc.sync.dma_start(xoS[:, :], xoT_d[:, :])
            FUSED1 = const.tile([128, NT, H], f32)
            FUSED2 = const.tile([128, NT, O], f32)
            for t in range(NT):
                psf = ps_bld.tile([128, H], f32, tag="psb")
                nc.tensor.matmul(psf[:, :], lhsT=xoS[:, t * 128:(t + 1) * 128],
                                 rhs=W1s[:, :], start=True, stop=True)
                nc.scalar.activation(FUSED1[:, t, :], psf[:, :], ACTF.Copy,
                                     scale=dinvS2[:, t:t + 1])
                nc.vector.tensor_add(FUSED1[:, t, :], FUSED1[:, t, :],
                                     BIAS1[:, :])

            # ---------------- layer-1 message table: tab1_b = dinv*(x@W1)
            def build_table1(b):
                nodes_b = min(BLK, N - b * BLK)
                for s0 in range(0, nodes_b, SLAB):
                    w = min(SLAB, nodes_b - s0)
                    xs = xpool.tile([F, SLAB], f16, tag="xs")
                    nc.sync.dma_start(xs[:, :w],
                                      xT_d[:, b * BLK + s0: b * BLK + s0 + w])
                    st = t1pool.tile([128, _cdiv(SLAB, 128), H], f16, tag="t1st")
                    nfull = 0
                    for j0 in range(0, w, 128):
                        m = min(128, w - j0)
                        jt = (s0 + j0) // 128  # node-tile idx within block
                        ps = ps_bld.tile([128, H], f32, tag="psb")
                        nc.tensor.matmul(ps[:m, :], lhsT=xs[:, j0:j0 + m],
                                         rhs=W1s[:, :], start=True, stop=True)
                        nc.scalar.activation(
                            st[:m, j0 // 128, :], ps[:m, :], ACTF.Copy,
                            scale=dinvB[:m, b * NBT + jt: b * NBT + jt + 1])
                        if m == 128:
                            nfull += 1
                    # store staged tiles to DRAM
                    if nfull:
                        dst_ap = tab1[b][s0:s0 + nfull * 128, :H].rearrange(
                            "(j p) f -> p j f", p=128)
                        nc.sync.dma_start(dst_ap, st[:, :nfull, :])
                    if nfull * 128 < w:  # ragged tail tile of the block
                        m = w - nfull * 128
                        nc.sync.dma_start(
                            tab1[b][s0 + nfull * 128: s0 + w, :H],
                            st[:m, nfull, :])

            if "notab1" not in cfg.ABL:
                for b in range(NBLK):
                    build_table1(b)

            # ---------------- gather + segment-sum matmul for one layer
            MAXKB = 8  # S-matrices built per DVE instruction
            qrot = [0]  # SWDGE queue rotation across gather calls

            def seg_layer(table_aps, uw):
                """table_aps[b]: block b's [rows, TW] f16 message rows; only
                the first uw columns are meaningful."""
                if "noseg" in cfg.ABL:
                    return
                for b in range(NBLK):
                    kt_in_blk = blk_kt_base[b + 1] - blk_kt_base[b]
                    if kt_in_blk == 0:
                        continue
                    # gather chunks
                    gtiles = []
                    for ci in range(_cdiv(kt_in_blk, CH_KT)):
                        kts = min(CH_KT, kt_in_blk - ci * CH_KT)
                        g = gpool.tile([128, CH_KT, TW], f16, tag="g")
                        tok0 = (blk_kt_base[b] + ci * CH_KT) * 128
                        if "nogather" not in cfg.ABL:
                            nc.gpsimd.dma_gather(
                                g[:, :kts, :], table_aps[b],
                                idxS[:, tok0 // 16: (tok0 + kts * 128) // 16],
                                num_idxs=kts * 128, num_idxs_reg=kts * 128,
                                elem_size=TW, single_packet=False,
                                queue_num=qrot[0] % 4)
                            qrot[0] += 1
                        else:
                            nc.vector.memset(g[:, :kts, :], 0.0)
                        gtiles.append(g)

                    def gslice(kglob):
                        ci, sl = divmod(kglob - blk_kt_base[b], CH_KT)
                        return gtiles[ci][:, sl, :uw]

                    # consume: per dst-tile, its token range [o0, o1) in the
                    # common layout; K-tiles at supertile boundaries are
                    # shared between adjacent dst-tiles (S masks the others).
                    for t in range(NT):
                        if "nosmm" in cfg.ABL:
                            break
                        ct = int(lay.ctok[b, t])
                        if ct == 0:
                            continue
                        o0 = int(lay.off[b, t])
                        o1 = o0 + ct
                        k0, k1 = o0 // 128, (o1 - 1) // 128
                        it = t % GT  # iota variant within supertile
                        ps = ps_seg.tile([128, uw], f32, tag="pss")
                        k = k0
                        while k <= k1:
                            kb = min(MAXKB, k1 + 1 - k)
                            Sb = spool.tile([128, MAXKB, 128], f16, tag="s")
                            nc.vector.tensor_tensor(
                                out=Sb[:, :kb, :],
                                in0=IOTA16[:, it * 128:(it + 1) * 128]
                                    .unsqueeze(1)
                                    .broadcast_to([128, kb, 128]),
                                in1=dloc[:, k:k + kb].unsqueeze(2)
                                    .broadcast_to([128, kb, 128]),
                                op=ALU.is_equal)
                            for j in range(kb):
                                nc.tensor.matmul(
                                    ps[:, :], lhsT=Sb[:, j, :],
                                    rhs=gslice(k + j),
                                    start=(k + j == k0),
                                    stop=(k + j == k1))
                            k += kb
                        nc.vector.tensor_add(agg[:, t, :uw],
                                             agg[:, t, :uw], ps[:, :])

            # ---------------- layer 1
            tab1_aps = [tab1[b][:, :] for b in range(NBLK)]
            seg_layer(tab1_aps, H)

            # post: h1 = relu(dinv*agg + FUSED1); t2 = dinv*(h1@W2) pad 40->64
            # also captures FUSED2[:, t, :] = BIAS2 + dinv * t2row  and
            # AllGathers t2loc in AGCH chunks (first chunks overlap later grps)
            def staged_store(dram, stile, grp, nt_in_grp, width):
                """store staging tile rows [grp*STG .. ) handling ragged tail"""
                t0 = grp * STG
                nfull = 0
                for tt in range(nt_in_grp):
                    if (t0 + tt) * 128 + 128 <= SHARD:
                        nfull += 1
                if nfull:
                    dst = dram[t0 * 128: t0 * 128 + nfull * 128,
                               :width].rearrange("(j p) f -> p j f", p=128)
                    nc.sync.dma_start(dst, stile[:, :nfull, :width])
                if nfull < nt_in_grp:
                    nc.sync.dma_start(
                        dram[(t0 + nfull) * 128: SHARD, :width],
                        stile[:LASTV, nfull, :width])

            ngrp = _cdiv(NT, STG)
            # AllGather chunk boundaries in units of post groups
            ag_after = set()
            rows_done = []
            per = _cdiv(ngrp, cfg.AGCH)
            gb = 0
            prev_row = 0
            while gb < ngrp:
                ge = min(gb + per, ngrp)
                row_end = min(ge * STG * 128, SHARD)
                ag_after.add(ge - 1)
                rows_done.append((prev_row, row_end))
                prev_row = row_end
                gb = ge

            def issue_ag(r0, r1):
                if r0 == 0 and r1 == SHARD:
                    tab2_view = tab2[:, :]
                else:  # strided view: rejected by BIR verifier today
                    tab2_view = tab2[:, :].rearrange(
                        "(c s) w -> c s w", c=C)[:, r0:r1, :]
                nc.gpsimd.collective_compute(
                    "AllGather", mybir.AluOpType.bypass,
                    replica_groups=[list(range(C))],
                    ins=[t2loc[r0:r1, :].opt()],
                    outs=[tab2_view.opt()])

            agq = list(rows_done)
            for grp in range(ngrp):
                nt_in_grp = min(STG, NT - grp * STG)
                st = post.tile([128, STG, H], f16, tag="t2st")
                if H > O:
                    nc.vector.memset(st[:, :, O:], 0.0)
                for tt in range(nt_in_grp):
                    if "nopost1" in cfg.ABL:
                        break
                    t = grp * STG + tt
                    h1 = work.tile([128, H], f32, tag="h1")
                    nc.vector.scalar_tensor_tensor(
                        out=h1[:, :], in0=agg[:, t, :],
                        scalar=dinvS[:, t:t + 1], in1=FUSED1[:, t, :],
                        op0=ALU.mult, op1=ALU.add)
                    nc.scalar.activation(h1[:, :], h1[:, :], ACTF.Relu)
                    pst = ps_tr.tile([H, 128], f32, tag="pstr")
                    nc.tensor.transpose(pst[:, :], h1[:, :], ID[:, :])
                    h1t = work.tile([H, 128], f32, tag="h1t")
                    nc.scalar.copy(h1t[:, :], pst[:, :])
                    ps2 = ps_t2.tile([128, O], f32, tag="pst2")
                    nc.tensor.matmul(ps2[:, :], lhsT=h1t[:, :], rhs=W2s[:, :],
                                     start=True, stop=True)
                    nc.scalar.activation(st[:, tt, :O], ps2[:, :], ACTF.Copy,
                                         scale=dinvS[:, t:t + 1])
                    # fused layer-2 bias: BIAS2 + dinv * t2row
                    nc.scalar.activation(FUSED2[:, t, :], ps2[:, :], ACTF.Copy,
                                         scale=dinvS2[:, t:t + 1])
                    nc.vector.tensor_add(FUSED2[:, t, :], FUSED2[:, t, :],
                                         BIAS2[:, :])
                staged_store(t2loc, st, grp, nt_in_grp, H)
                if grp in ag_after and "noag" not in cfg.ABL and agq:
                    r0, r1 = agq.pop(0)
                    issue_ag(r0, r1)

            # ---------------- layer 2
            nc.vector.memset(agg[:, :, :], 0.0)
            tab2_aps = [tab2[b * BLK: b * BLK + min(BLK, N - b * BLK), :]
                        for b in range(NBLK)]
            seg_layer(tab2_aps, O)

            # post: y = relu(dinv*agg + FUSED2); out = log_softmax(y)
            for grp in range(ngrp):
                nt_in_grp = min(STG, NT - grp * STG)
                st = opool.tile([128, STG, O], f32, tag="ost")
                for tt in range(nt_in_grp):
                    if "nopost2" in cfg.ABL:
                        break
                    t = grp * STG + tt
                    y = work.tile([128, O], f32, tag="y")
                    nc.vector.scalar_tensor_tensor(
                        out=y[:, :], in0=agg[:, t, :O],
                        scalar=dinvS[:, t:t + 1], in1=FUSED2[:, t, :],
                        op0=ALU.mult, op1=ALU.add)
                    nc.scalar.activation(y[:, :], y[:, :], ACTF.Relu)
                    nmax = work.tile([128, 1], f32, tag="nmax")
                    nc.vector.tensor_reduce(nmax[:, :], y[:, :],
                                            axis=mybir.AxisListType.X,
                                            op=ALU.max, negate=True)
                    ex = work.tile([128, O], f32, tag="ex")
                    esum = work.tile([128, 1], f32, tag="esum")
                    nc.scalar.activation(ex[:, :], y[:, :], ACTF.Exp,
                                         bias=nmax[:, :], scale=1.0,
                                         accum_out=esum[:, :])
                    lsum = work.tile([128, 1], f32, tag="lsum")
                    nc.scalar.activation(lsum[:, :], esum[:, :], ACTF.Ln)
                    nc.vector.tensor_scalar(
                        out=st[:, tt, :], in0=y[:, :], scalar1=nmax[:, :],
                        scalar2=lsum[:, :], op0=ALU.add, op1=ALU.subtract)
                staged_store(out_d, st, grp, nt_in_grp, O)

    nc.compile()
    return nc


# ------------------------------------------------------------------ entry --
def prepare_and_run(inputs, cfg=None, trace=False, **run_kwargs):
    """Preprocess, build, run on 8 cores.  Returns (out, BassKernelResults)."""
    from concourse.bass_utils import run_bass_kernel_spmd

    cfg = cfg or CFG()
    lay, in_maps = host_prepare(inputs, cfg)
    nc = _build(cfg, lay)
    res = run_bass_kernel_spmd(nc, in_maps, core_ids=list(range(cfg.C)),
                               trace=trace, **run_kwargs)
    out = np.concatenate([res.results[c]["out"] for c in range(cfg.C)], axis=0)
    return out.astype(np.float32), res


def kernel(**inputs):
    out, _ = prepare_and_run(inputs)
    return out


if __name__ == "__main__":
    import reference

    inputs = {k: np.asarray(v) for k, v in reference.setup_inputs().items()}
    got = kernel(**inputs)
    want = np.asarray(reference.reference(**inputs))
    err = np.abs(got - want).max() / max(np.abs(want).max(), 1e-9)
    print("rel err:", err)
